# revision 8
# baseline (speedup 1.0000x reference)
"""Sliding-window GQA attention with RoPE on 8 trn2 NeuronCores.

Sharding: core c = (b, g) with b = c // 4 (batch), g = c % 4 (kv-head group).
Each core computes its 4 query heads + 1 kv head for one batch element and
produces a partial output (its head-group's contribution to x @ Wo); the host
sums the 4 partials per batch.

All matmuls run in float32r (tf32-like, full PE rate at N>=256).
Layout strategy: activations kept transposed ([feature, seq]) so that
projections, scores (S^T), AV (U^T) and the output projection all consume
naturally-laid-out operands; softmax normalization is deferred to after AV.
"""

import math

import numpy as np

B, S, E = 2, 2048, 2048
H, KV, D = 16, 4, 128
WIN = 512
THETA = 10000.0
SCALE = 1.0 / math.sqrt(D)
NCORES = 8
GH = H // KV          # 4 query heads per kv group
MG = GH * D           # 512 q-features per group
ET = E // 128         # 16 e-tiles
ST = S // 128         # 16 s-tiles
NSC = S // 512        # 4 s-chunks / q-chunks

_CACHE = {}


def _build_module():
    import concourse.bacc as bacc
    import concourse.tile as tile
    import concourse.mybir as mybir

    F32R = mybir.dt.float32r
    F32 = mybir.dt.float32
    EXP = mybir.ActivationFunctionType.Exp

    nc = bacc.Bacc("TRN2", target_bir_lowering=False, debug=False,
                   enable_asserts=False, num_devices=NCORES)

    x = nc.dram_tensor("x", [S, E], F32R, kind="ExternalInput").ap()
    wq = nc.dram_tensor("wq", [E, MG], F32R, kind="ExternalInput").ap()
    wk = nc.dram_tensor("wk", [E, D], F32R, kind="ExternalInput").ap()
    wv = nc.dram_tensor("wv", [E, D], F32R, kind="ExternalInput").ap()
    wo = nc.dram_tensor("wo", [MG, E], F32R, kind="ExternalInput").ap()
    cos2 = nc.dram_tensor("cos2", [128, S], F32, kind="ExternalInput").ap()
    ssin = nc.dram_tensor("ssin", [128, S], F32, kind="ExternalInput").ap()
    m1 = nc.dram_tensor("m1", [128, 128], F32R, kind="ExternalInput").ap()
    m2 = nc.dram_tensor("m2", [128, 128], F32R, kind="ExternalInput").ap()
    ones = nc.dram_tensor("ones", [128, 1], F32R, kind="ExternalInput").ap()
    ident = nc.dram_tensor("ident", [128, 128], F32R, kind="ExternalInput").ap()
    zeros = nc.dram_tensor("zeros", [128, 512], F32R, kind="ExternalInput").ap()
    out = nc.dram_tensor("out", [S, E], F32, kind="ExternalOutput").ap()
    rscr = nc.dram_tensor("rscr", [GH * NSC, 512], F32, kind="Internal").ap()

    with tile.TileContext(nc) as tc:
        with (
            tc.tile_pool(name="persist", bufs=1) as pp,
            tc.tile_pool(name="consts", bufs=1) as cp,
        ):
            # Persistent T-layout activations
            qt = pp.tile([128, GH, S], F32R, tag="qt")       # Q^T per head
            kt = pp.tile([128, S], F32R, tag="kt")           # K^T
            vt = pp.tile([128, S], F32R, tag="vt")           # V^T

            wq_sb = cp.tile([128, ET, MG], F32R, tag="wq")
            wk_sb = cp.tile([128, ET, D], F32R, tag="wk")
            wv_sb = cp.tile([128, ET, D], F32R, tag="wv")
            cos_sb = cp.tile([128, S], F32, tag="cos")
            sin_sb = cp.tile([128, S], F32, tag="sin")
            m1_sb = cp.tile([128, 128], F32R, tag="m1")
            m2_sb = cp.tile([128, 128], F32R, tag="m2")
            ones_sb = cp.tile([128, 1], F32R, tag="ones")
            id_sb = cp.tile([128, 128], F32R, tag="id")
            z_sb = cp.tile([128, 512], F32R, tag="z")

            nc.sync.dma_start(wq_sb[:], wq.rearrange("(t p) m -> p t m", p=128))
            nc.sync.dma_start(wk_sb[:], wk.rearrange("(t p) m -> p t m", p=128))
            nc.sync.dma_start(wv_sb[:], wv.rearrange("(t p) m -> p t m", p=128))
            nc.sync.dma_start(cos_sb[:], cos2[:])
            nc.sync.dma_start(sin_sb[:], ssin[:])
            nc.sync.dma_start(m1_sb[:], m1[:])
            nc.sync.dma_start(m2_sb[:], m2[:])
            nc.sync.dma_start(ones_sb[:], ones[:])
            nc.sync.dma_start(id_sb[:], ident[:])
            nc.sync.dma_start(z_sb[:], zeros[:])

            # ---------------- Phase A: projections + RoPE ----------------
            with (
                tc.tile_pool(name="xin", bufs=2) as xp,
                tc.tile_pool(name="xT", bufs=4) as xtp,
                tc.tile_pool(name="rope", bufs=2) as rp,
                tc.tile_pool(name="psA", bufs=1, space="PSUM") as psa,
                tc.tile_pool(name="psT", bufs=2, space="PSUM") as pst,
            ):
                for sc in range(NSC):
                    s0 = sc * 512
                    xin = xp.tile([128, 4, E], F32R, tag="xin")
                    for si in range(4):
                        nc.sync.dma_start(xin[:, si, :], x[s0 + si * 128: s0 + (si + 1) * 128, :])

                    q_ps = [psa.tile([128, 512], F32, name=f"qps{h}", tag=f"qps{h}")
                            for h in range(GH)]
                    k_ps = psa.tile([128, 512], F32, tag="kps")
                    v_ps = psa.tile([128, 512], F32, tag="vps")

                    for et in range(ET):
                        xt_et = xtp.tile([128, 512], F32R, tag="xT")
                        for si in range(4):
                            tp_ps = pst.tile([128, 128], F32R, tag="tp")
                            nc.tensor.transpose(
                                tp_ps[:], xin[:, si, et * 128:(et + 1) * 128], id_sb[:])
                            nc.vector.tensor_copy(xt_et[:, si * 128:(si + 1) * 128], tp_ps[:])
                        first, last = et == 0, et == ET - 1
                        for h in range(GH):
                            nc.tensor.matmul(
                                q_ps[h][:], wq_sb[:, et, h * 128:(h + 1) * 128],
                                xt_et[:], start=first, stop=last)
                        nc.tensor.matmul(k_ps[:], wk_sb[:, et, :], xt_et[:],
                                         start=first, stop=last)
                        nc.tensor.matmul(v_ps[:], wv_sb[:, et, :], xt_et[:],
                                         start=first, stop=last)

                    # RoPE: rot(p) = p*cos + swap_halves(p)*sgn_sin
                    for h in range(GH + 1):
                        src = q_ps[h] if h < GH else k_ps
                        dst = qt[:, h, s0:s0 + 512] if h < GH else kt[:, s0:s0 + 512]
                        a_t = rp.tile([128, 512], F32, tag="ropeA")
                        b_t = rp.tile([128, 512], F32, tag="ropeB")
                        nc.vector.tensor_mul(a_t[:], src[:], cos_sb[:, s0:s0 + 512])
                        nc.vector.tensor_mul(
                            b_t[0:64, :], src[64:128, :], sin_sb[0:64, s0:s0 + 512])
                        nc.vector.tensor_mul(
                            b_t[64:128, :], src[0:64, :], sin_sb[64:128, s0:s0 + 512])
                        nc.vector.tensor_add(dst, a_t[:], b_t[:])
                    nc.vector.tensor_copy(vt[:, s0:s0 + 512], v_ps[:])

            # persist2: tensors whose lifetime starts after phase A
            pp2_cm = tc.tile_pool(name="persist2", bufs=1)
            pp2 = pp2_cm.__enter__()
            vn = pp2.tile([128, ST, 128], F32R, tag="vn")    # V natural, k-tiled
            ot = pp2.tile([128, GH, S], F32R, tag="ot")      # O^T per head
            wo_sb = pp2.tile([128, GH, E], F32R, tag="wo")
            nc.sync.dma_start(wo_sb[:], wo.rearrange("(f p) e -> p f e", p=128))

            # Phase B: V^T -> V natural (k on partitions)
            with tc.tile_pool(name="psB", bufs=2, space="PSUM") as psb:
                for t in range(ST):
                    tp_ps = psb.tile([128, 128], F32R, tag="tp")
                    nc.tensor.transpose(tp_ps[:], vt[:, t * 128:(t + 1) * 128], id_sb[:])
                    nc.vector.tensor_copy(vn[:, t, :], tp_ps[:])

            # ---------------- Phase C: sliding-window attention ----------------
            with (
                tc.tile_pool(name="att", bufs=6) as ap_,
                tc.tile_pool(name="norm", bufs=2) as np_,
                tc.tile_pool(name="psS", bufs=3, space="PSUM") as pss,
                tc.tile_pool(name="psU", bufs=2, space="PSUM") as psu,
                tc.tile_pool(name="psR", bufs=2, space="PSUM") as psr,
            ):
                for h in range(GH):
                    for qc in range(NSC):
                        i0 = qc * 4
                        c0 = qc * 512
                        ts_ = list(range(max(0, i0 - 4), i0 + 4))
                        ut_ps = psu.tile([128, 512], F32, tag="ut")
                        r_ps = psr.tile([1, 512], F32, tag="r")
                        for idx, t in enumerate(ts_):
                            ilo, ihi = max(i0, t), min(i0 + 3, t + 4)
                            vlo, vhi = (ilo - i0) * 128, (ihi - i0 + 1) * 128
                            clo, chi = vlo, vhi
                            if chi - clo < 256:
                                if clo >= 128:
                                    clo -= 128
                                else:
                                    chi += 128
                            st_ps = pss.tile([128, 512], F32, tag="st")
                            nc.tensor.matmul(
                                st_ps[:, clo:chi], kt[:, t * 128:(t + 1) * 128],
                                qt[:, h, c0 + clo:c0 + chi], start=True, stop=True)
                            at = ap_.tile([128, 512], F32R, tag="at")
                            nc.scalar.activation(
                                at[:, vlo:vhi], st_ps[:, vlo:vhi], EXP, scale=SCALE)
                            if t >= i0:  # causal (diagonal) mask at q-tile i = t
                                bnd = (t - i0) * 128
                                nc.vector.tensor_mul(
                                    at[:, bnd:bnd + 128], at[:, bnd:bnd + 128], m2_sb[:])
                            if i0 <= t + 4 <= i0 + 3:  # window-edge mask at i = t+4
                                bnd = (t + 4 - i0) * 128
                                nc.vector.tensor_mul(
                                    at[:, bnd:bnd + 128], at[:, bnd:bnd + 128], m1_sb[:])
                            if vlo > 0:
                                nc.gpsimd.tensor_copy(at[:, 0:vlo], z_sb[:, 0:vlo])
                            if vhi < 512:
                                nc.gpsimd.tensor_copy(at[:, vhi:512], z_sb[:, vhi:512])
                            first, last = idx == 0, idx == len(ts_) - 1
                            nc.tensor.matmul(ut_ps[:], vn[:, t, :], at[:],
                                             start=first, stop=last)
                            nc.tensor.matmul(r_ps[:1, :], ones_sb[:], at[:],
                                             start=first, stop=last)
                        rinv = np_.tile([1, 512], F32, tag="rinv")
                        nc.vector.reciprocal(rinv[:1, :], r_ps[:1, :])
                        hq = h * NSC + qc
                        nc.sync.dma_start(rscr[hq:hq + 1, :], rinv[:1, :])
                        rrep = np_.tile([128, 512], F32, tag="rrep")
                        import concourse.bass as bass_mod
                        row = rscr[hq:hq + 1, :]
                        bcast = bass_mod.AP(
                            tensor=row.tensor, offset=row.offset,
                            ap=[[0, 128]] + [list(p) for p in row.ap[1:]])
                        nc.sync.dma_start(rrep[:], bcast)
                        nc.vector.tensor_mul(ot[:, h, c0:c0 + 512], ut_ps[:], rrep[:])

            # ---------------- Phase D: output projection ----------------
            with (
                tc.tile_pool(name="osb", bufs=3) as op_,
                tc.tile_pool(name="psO", bufs=3, space="PSUM") as pso,
            ):
                for st_i in range(ST):
                    r0 = st_i * 128
                    for eo in range(4):
                        e0 = eo * 512
                        o_ps = pso.tile([128, 512], F32, tag="ops")
                        for f in range(GH):
                            nc.tensor.matmul(
                                o_ps[:], ot[:, f, r0:r0 + 128],
                                wo_sb[:, f, e0:e0 + 512],
                                start=(f == 0), stop=(f == GH - 1))
                        o_sb = op_.tile([128, 512], F32, tag="osb")
                        nc.vector.tensor_copy(o_sb[:], o_ps[:])
                        nc.sync.dma_start(out[r0:r0 + 128, e0:e0 + 512], o_sb[:])

            pp2_cm.__exit__(None, None, None)

    nc.compile()
    return nc


def _host_constants():
    pos = np.arange(S, dtype=np.float64)
    inv = 1.0 / (THETA ** (np.arange(0, D, 2, dtype=np.float64) / D))  # [64]
    ang = inv[:, None] * pos[None, :]                                   # [64, S]
    cos2 = np.concatenate([np.cos(ang), np.cos(ang)], 0).astype(np.float32)
    ssin = np.concatenate([-np.sin(ang), np.sin(ang)], 0).astype(np.float32)
    jj = np.arange(128)[:, None]
    qq = np.arange(128)[None, :]
    m1 = (jj >= qq + 1).astype(np.float32)
    m2 = (jj <= qq).astype(np.float32)
    ones = np.ones((128, 1), np.float32)
    ident = np.eye(128, dtype=np.float32)
    zeros = np.zeros((128, 512), np.float32)
    return cos2, ssin, m1, m2, ones, ident, zeros


def _get_runner():
    if "run" in _CACHE:
        return _CACHE["run"]
    import jax
    import jax.numpy as jnp
    from jax.sharding import Mesh, PartitionSpec
    from jax.experimental.shard_map import shard_map
    import concourse.mybir as mybir_m
    from concourse.bass2jax import _bass_exec_p, install_neuronx_cc_hook, partition_id_tensor

    install_neuronx_cc_hook()
    nc = _build_module()

    partition_name = nc.partition_id_tensor.name if nc.partition_id_tensor else None
    in_names, out_names, out_avals, out_shapes = [], [], [], []
    for alloc in nc.m.functions[0].allocations:
        if not isinstance(alloc, mybir_m.MemoryLocationSet):
            continue
        name = alloc.memorylocations[0].name
        if alloc.kind == "ExternalInput":
            if name != partition_name:
                in_names.append(name)
        elif alloc.kind == "ExternalOutput":
            out_names.append(name)
            shape = tuple(alloc.tensor_shape)
            dtype = mybir_m.dt.np(alloc.dtype)
            out_avals.append(jax.core.ShapedArray(shape, dtype))
            out_shapes.append((shape, dtype))
    n_params = len(in_names)
    all_names = list(in_names) + out_names
    if partition_name is not None:
        all_names.append(partition_name)
    donate = tuple(range(n_params, n_params + len(out_names)))

    def _body(*args):
        operands = list(args)
        if partition_name is not None:
            operands.append(partition_id_tensor())
        outs = _bass_exec_p.bind(
            *operands,
            out_avals=tuple(out_avals),
            in_names=tuple(all_names),
            out_names=tuple(out_names),
            lowering_input_output_aliases=(),
            sim_require_finite=False,
            sim_require_nnan=False,
            nc=nc,
        )
        return tuple(outs)

    devices = jax.devices()[:NCORES]
    mesh = Mesh(np.asarray(devices), ("core",))
    in_specs = (PartitionSpec("core"),) * (n_params + len(out_names))
    out_specs = (PartitionSpec("core"),) * len(out_names)
    jf = jax.jit(
        shard_map(_body, mesh=mesh, in_specs=in_specs, out_specs=out_specs,
                  check_rep=False),
        donate_argnums=donate, keep_unused=True)

    zshapes = [( (NCORES * sh[0],) + tuple(sh[1:]), dt) for sh, dt in out_shapes]
    mkzeros = jax.jit(lambda: tuple(jnp.zeros(s, d) for s, d in zshapes))

    def put_inputs(in_maps):
        concat_in = [
            np.concatenate([np.asarray(in_maps[c][n]) for c in range(NCORES)], axis=0)
            for n in in_names
        ]
        return [jax.device_put(a) for a in concat_in]

    def execute(ins_dev):
        zeros = mkzeros()
        out_arrs = jf(*ins_dev, *zeros)
        jax.block_until_ready(out_arrs)
        return out_arrs

    def fetch(out_arrs):
        res = []
        hostized = [np.asarray(a).reshape((NCORES,) + out_shapes[i][0])
                    for i, a in enumerate(out_arrs)]
        for c in range(NCORES):
            res.append({nme: hostized[i][c] for i, nme in enumerate(out_names)})
        return res

    def runner(in_maps):
        return fetch(execute(put_inputs(in_maps)))

    runner.put_inputs = put_inputs
    runner.execute = execute
    runner.fetch = fetch
    _CACHE["run"] = runner
    return runner


def _make_in_maps(x_full, Wq, Wk, Wv, Wo):
    cos2, ssin, m1, m2, ones, ident, zeros = _CACHE.setdefault("consts", _host_constants())
    in_maps = []
    for c in range(NCORES):
        b, g = c // KV, c % KV
        in_maps.append({
            "x": np.ascontiguousarray(x_full[b], np.float32),
            "wq": np.ascontiguousarray(Wq[:, g * MG:(g + 1) * MG], np.float32),
            "wk": np.ascontiguousarray(Wk[:, g * D:(g + 1) * D], np.float32),
            "wv": np.ascontiguousarray(Wv[:, g * D:(g + 1) * D], np.float32),
            "wo": np.ascontiguousarray(Wo[g * MG:(g + 1) * MG, :], np.float32),
            "cos2": cos2, "ssin": ssin, "m1": m1, "m2": m2,
            "ones": ones, "ident": ident, "zeros": zeros,
        })
    return in_maps


def kernel(x, Wq, Wk, Wv, Wo):
    x = np.asarray(x, np.float32)
    run = _get_runner()
    res = run(_make_in_maps(x, np.asarray(Wq), np.asarray(Wk), np.asarray(Wv),
                            np.asarray(Wo)))
    out = np.zeros((B, S, E), np.float32)
    for c in range(NCORES):
        out[c // KV] += res[c]["out"]
    return out
